# revision 12
# baseline (speedup 1.0000x reference)
"""Trainium2 Bass kernel for masked edge pooling + linear (nn_EtoX).

Reference computation (per sample b, node i, over neighbors j with mask[b, j]):
  m   = sum_j E[b,i,j,:] / count_b          (unmasked sum / masked count)
  mi  = min over present j of E[b,i,j,:]
  ma  = max over present j of E[b,i,j,:]
  std = sum_{present j} (E - m)^2 / count_b
  out = concat(m, mi, ma, std) @ W.T + bias

Strategy (v2): data-parallel over batch (2 samples per core, 8 cores). The
host permutes each sample's j axis so present rows come first (padded to 256
with duplicates of the first present row), then absent rows (padded to CA
with duplicates of the first absent row), and uploads the result in FP16 —
halving HBM traffic and removing all on-device casts.

On-device per (sample, 128-row i-block), everything is pairwise fp16
tensor_tensor trees on the Vector engine (2x perf mode):
  - min/max trees over the 256 present rows
  - sum tree over present rows (pad contribution subtracted exactly)
  - ScalarE squares the present tile; sum tree over squares gives sumsq
  - sum tree over the CA absent rows (for the unmasked-mean correction)
  - small fused epilogue algebra forms m and std; TensorE transposes the
    packed z halves and applies the 256x256 linear (+bias) in fp16
"""

import os

# Whole-tile dependency granularity: lets a 1-element ACT "fence" write
# supersede a DMA-landed tile's reader/writer dep set, keeping every DMA
# instruction within the hardware's 2-sync-wait budget.
os.environ.setdefault("BY_DEFAULT_DISABLE_SUBTILE_DEPS", "1")

import numpy as np

try:
    from concourse import bass, mybir, tile
    from concourse.bass_utils import run_bass_kernel_spmd
except ImportError:  # fall back to the container's repo checkout
    import sys

    sys.path.insert(0, "/opt/trn_rl_repo")
    from concourse import bass, mybir, tile
    from concourse.bass_utils import run_bass_kernel_spmd

BS, N, DE, DX = 16, 256, 64, 256
FI = 4 * DE
NCORES = 8
BPC = BS // NCORES  # samples per core
P = 128
NBLK = BPC * 2  # 128-row i-blocks per core

F32 = mybir.dt.float32
F16 = mybir.dt.float16

LAST_RESULT = {}

_NC_CACHE = {}


def _enable_tracing():
    """Install the NTFF profile hook that the image's ``antenv`` lacks."""
    import contextlib
    import ctypes
    import sys
    import types

    try:
        import antenv.axon_hooks  # noqa: F401

        pass
    except ImportError:
        so_path = "/opt/axon/libaxon_pjrt.so"
        lib = ctypes.CDLL(so_path)
        if hasattr(lib, "axon_start_nrt_profile"):
            lib.axon_start_nrt_profile.argtypes = [
                ctypes.POINTER(ctypes.c_int64),
                ctypes.c_size_t,
            ]
            lib.axon_start_nrt_profile.restype = ctypes.c_int64
            lib.axon_stop_nrt_profile.argtypes = [ctypes.c_char_p]
            lib.axon_stop_nrt_profile.restype = ctypes.c_int64

            @contextlib.contextmanager
            def _hook(output_dir, device_ids):
                import jax

                jax.devices()
                if device_ids:
                    ids = (ctypes.c_int64 * len(device_ids))(*device_ids)
                    rc = lib.axon_start_nrt_profile(ids, len(device_ids))
                else:
                    rc = lib.axon_start_nrt_profile(None, 0)
                if rc != 0:
                    raise RuntimeError(f"axon_start_nrt_profile rc={rc}")
                try:
                    yield
                finally:
                    n = lib.axon_stop_nrt_profile(str(output_dir).encode())
                    print(f"profile: {n} file(s) written to {output_dir}")

            mod = types.ModuleType("antenv.axon_hooks")
            mod.get_axon_ntff_profile_hook = lambda: _hook
            mod.set_axon_ntff_profile_hook = lambda h: None
            import antenv

            sys.modules["antenv.axon_hooks"] = mod
            antenv.axon_hooks = mod

    from concourse import bass_utils as _bu

    _bu.upload_artifacts = lambda tmpdir: f"file://{tmpdir}"


def _hoist_excess_waits(bir: dict) -> dict:
    """Walrus (this build) rejects instructions whose embedded sync-wait list
    exceeds the ISA struct's slots. Hoist all but one wait into standalone
    single-wait EventSemaphore instructions placed immediately before the
    instruction on the same engine stream - semantically identical (sequencer
    executes waits in stream order before the op)."""
    ctr = 0
    for fn in bir["functions"]:
        for blk in fn["blocks"]:
            new = []
            for ins in blk["instructions"]:
                si = ins.get("sync_info")
                if si:
                    waits = si.get("on_wait") or []
                    if len(waits) > 1:
                        for w in waits[:-1]:
                            ctr += 1
                            new.append(
                                {
                                    "debug": ins.get("debug", 0),
                                    "engine": ins["engine"],
                                    "ins": [],
                                    "outs": [],
                                    "name": f"hoistw-{ctr}",
                                    "opcode": "EventSemaphore",
                                    "sync_info": {"on_update": [], "on_wait": [w]},
                                }
                            )
                        si["on_wait"] = [waits[-1]]
                new.append(ins)
            blk["instructions"] = new
    return bir


def build_program(CA: int) -> "bass.Bass":
    nc = bass.Bass()
    NI = BPC * N  # flattened (sample, i) rows
    eg = nc.declare_dram_parameter("eg", [NI, N + CA, DE], F16, isOutput=False)
    wt = nc.declare_dram_parameter("wt", [FI, DX], F16, isOutput=False)
    brow = nc.declare_dram_parameter("brow", [1, DX], F16, isOutput=False)
    ident = nc.declare_dram_parameter("ident", [P, P], F16, isOutput=False)
    scal = nc.declare_dram_parameter("scal", [NBLK, P, 3], F32, isOutput=False)
    out = nc.declare_dram_parameter("out", [NI, DX], F32, isOutput=True)

    MIN = mybir.AluOpType.min
    MAX = mybir.AluOpType.max
    ADD = mybir.AluOpType.add
    SUB = mybir.AluOpType.subtract
    MUL = mybir.AluOpType.mult
    SQUARE = mybir.ActivationFunctionType.Square

    with tile.TileContext(nc) as tc:
        with (
            tc.tile_pool(name="singles", bufs=1) as singles,
            tc.tile_pool(name="main", bufs=2) as mainp,
            tc.tile_pool(name="absp", bufs=2) as absp,
            tc.tile_pool(name="sqp", bufs=1) as sqp,
            tc.tile_pool(name="scr", bufs=1) as scr,
            tc.tile_pool(name="stats", bufs=2) as stats,
            tc.tile_pool(name="ep", bufs=2) as ep,
            tc.tile_pool(name="outp", bufs=2) as outp,
            tc.tile_pool(name="psum", bufs=2, space="PSUM") as psum,
        ):
            wt0 = singles.tile([P, DX], F16, tag="wt0")
            nc.sync.dma_start(out=wt0[:], in_=wt[0:P, :])
            wt1 = singles.tile([P, DX], F16, tag="wt1")
            nc.sync.dma_start(out=wt1[:], in_=wt[P:FI, :])
            id_t = singles.tile([P, P], F16, tag="id")
            nc.sync.dma_start(out=id_t[:], in_=ident[:, :])
            br_t = singles.tile([1, DX], F16, tag="br")
            nc.sync.dma_start(out=br_t[:], in_=brow[:, :])
            ones1 = singles.tile([1, P], F16, tag="ones")
            nc.vector.memset(ones1[:], 1.0)
            sc = {}
            for b in range(NBLK):
                for k, nm in enumerate(("npadP", "npadA", "negInvC")):
                    t = singles.tile([P, 1], F32, tag=f"sc{b}{k}")
                    nc.sync.dma_start(out=t[:], in_=scal[b, :, k : k + 1])
                    sc[(b, nm)] = t

            # per-engine tree scratch (separate so DVE and GPSIMD chains
            # don't false-serialize on whole-tile deps)
            scratch = {}
            gscratch = {}
            for w in (128, 64, 32, 16, 8, 4, 2):
                scratch[w] = scr.tile([P, w, DE], F16, tag=f"t{w}", name=f"t{w}")
                gscratch[w] = scr.tile([P, w, DE], F16, tag=f"g{w}", name=f"g{w}")

            def chain(op, a_ap, b_ap, w1, final_dst, eng=None, sdict=None):
                """Pairwise reduce: L1 combines a_ap/b_ap ([P, w1, DE] each),
                then halve until one [P, DE] row written to final_dst."""
                eng = eng or nc.vector
                sdict = sdict or scratch
                cur = sdict[w1]
                eng.tensor_tensor(cur[:], a_ap, b_ap, op)
                w = w1 // 2
                while w >= 1:
                    if w == 1:
                        eng.tensor_tensor(
                            final_dst,
                            cur[:, 0:1, :].rearrange("p a d -> p (a d)"),
                            cur[:, 1:2, :].rearrange("p a d -> p (a d)"),
                            op,
                        )
                    else:
                        eng.tensor_tensor(
                            sdict[w][:], cur[:, 0:w, :], cur[:, w : 2 * w, :], op
                        )
                        cur = sdict[w]
                    w //= 2

            for b in range(NBLK):
                s, ih = b // 2, b % 2
                r0 = s * N + ih * P
                mainA = mainp.tile([P, P, DE], F16, tag="mainA")
                nc.sync.dma_start(out=mainA[:], in_=eg[r0 : r0 + P, 0:P, :])
                mainB = mainp.tile([P, P, DE], F16, tag="mainB")
                nc.sync.dma_start(out=mainB[:], in_=eg[r0 : r0 + P, P:N, :])
                abst = absp.tile([P, CA, DE], F16, tag="abs")
                nc.sync.dma_start(out=abst[:], in_=eg[r0 : r0 + P, N : N + CA, :])
                sqA = sqp.tile([P, P, DE], F16, tag="sqA")
                nc.scalar.activation(sqA[:], mainA[:], SQUARE)
                sqB = sqp.tile([P, P, DE], F16, tag="sqB")
                nc.scalar.activation(sqB[:], mainB[:], SQUARE)

                z01 = stats.tile([P, P], F16, tag="z01")  # [m | mi]
                z23 = stats.tile([P, P], F16, tag="z23")  # [ma | std]
                SPt = stats.tile([P, DE], F32, tag="SP")
                QPt = stats.tile([P, DE], F32, tag="QP")
                SAt = stats.tile([P, DE], F32, tag="SA")

                a_h, b_h = mainA[:, :, :], mainB[:, :, :]
                chain(MIN, a_h, b_h, P, z01[:, DE : 2 * DE])
                chain(MAX, a_h, b_h, P, z23[:, 0:DE])
                chain(ADD, a_h, b_h, P, SPt[:])
                # sumsq + absent-sum chains run on the (otherwise idle) GPSIMD
                chain(ADD, sqA[:, :, :], sqB[:, :, :], P, QPt[:],
                      eng=nc.gpsimd, sdict=gscratch)
                ch = CA // 2
                chain(ADD, abst[:, 0:ch, :], abst[:, ch:CA, :], ch, SAt[:],
                      eng=nc.gpsimd, sdict=gscratch)

                # epilogue: m = (S_p + S_a)/c ; std = (Q_p - m (S_p - S_a))/c
                # computed via negated partials so scalar_tensor_tensor fuses
                def et(tag):
                    return ep.tile([P, DE], F32, tag=tag, name=tag)

                x0 = mainA[:, 0, :]
                x0sq = sqA[:, 0, :]
                a0 = abst[:, 0, :]
                nSPc = et("nSPc")  # npadP*x0 - SP_tree = -S_p
                nc.vector.scalar_tensor_tensor(
                    nSPc[:], x0, sc[(b, "npadP")][:], SPt[:], MUL, SUB
                )
                nQPc = et("nQPc")  # npadP*x0^2 - QP_tree = -Q_p
                nc.vector.scalar_tensor_tensor(
                    nQPc[:], x0sq, sc[(b, "npadP")][:], QPt[:], MUL, SUB
                )
                nSAc = et("nSAc")  # npadA*a0 - SA_tree = -S_a
                nc.vector.scalar_tensor_tensor(
                    nSAc[:], a0, sc[(b, "npadA")][:], SAt[:], MUL, SUB
                )
                nS = et("nS")
                nc.vector.tensor_tensor(nS[:], nSPc[:], nSAc[:], ADD)
                m_dst = z01[:, 0:DE]
                nc.vector.tensor_scalar(m_dst, nS[:], sc[(b, "negInvC")][:], None, MUL)
                d_t = et("d")  # S_p - S_a
                nc.vector.tensor_tensor(d_t[:], nSAc[:], nSPc[:], SUB)
                e_t = et("e")
                nc.vector.tensor_tensor(e_t[:], m_dst, d_t[:], MUL)
                f_t = et("f")  # -(Q_p - m(S_p - S_a))
                nc.vector.tensor_tensor(f_t[:], nQPc[:], e_t[:], ADD)
                nc.vector.tensor_scalar(
                    z23[:, DE : 2 * DE], f_t[:], sc[(b, "negInvC")][:], None, MUL
                )

                # fences: collapse each DMA-landed tile's reader set
                nc.scalar.mul(mainA[0:1, 0:1, 0:1], mainA[0:1, 0:1, 0:1], 0.0)
                nc.scalar.mul(mainB[0:1, 0:1, 0:1], mainB[0:1, 0:1, 0:1], 0.0)
                nc.scalar.mul(sqA[0:1, 0:1, 0:1], sqA[0:1, 0:1, 0:1], 0.0)
                nc.scalar.mul(sqB[0:1, 0:1, 0:1], sqB[0:1, 0:1, 0:1], 0.0)
                nc.scalar.mul(abst[0:1, 0:1, 0:1], abst[0:1, 0:1, 0:1], 0.0)

                # transpose packed stats into z^T layout ([feature, i])
                psz0 = psum.tile([P, P], F16, tag="psz0")
                nc.tensor.transpose(out=psz0[:], in_=z01[:], identity=id_t[:])
                psz1 = psum.tile([P, P], F16, tag="psz1")
                nc.tensor.transpose(out=psz1[:], in_=z23[:], identity=id_t[:])
                zT0 = outp.tile([P, P], F16, tag="zT0")
                nc.scalar.copy(out=zT0[:], in_=psz0[:])
                zT1 = outp.tile([P, P], F16, tag="zT1")
                nc.scalar.copy(out=zT1[:], in_=psz1[:])

                pso = psum.tile([P, DX], F32, tag="pso")
                nc.tensor.matmul(pso[:], zT0[:], wt0[:], start=True, stop=False)
                nc.tensor.matmul(pso[:], zT1[:], wt1[:], start=False, stop=False)
                nc.tensor.matmul(pso[:], ones1[:], br_t[:], start=False, stop=True)
                o_t = outp.tile([P, DX], F32, tag="o_t")
                nc.scalar.copy(out=o_t[:], in_=pso[:])
                nc.sync.dma_start(out=out[r0 : r0 + P, :], in_=o_t[:])

    import json as _json

    _orig_to_json = nc.to_json_bytes

    def _patched_to_json():
        return _json.dumps(_hoist_excess_waits(_json.loads(_orig_to_json()))).encode()

    nc.to_json_bytes = _patched_to_json
    return nc


def kernel(E, e_mask2, W, b):
    E = np.asarray(E, dtype=np.float32)
    mask = np.asarray(e_mask2).reshape(BS, N).astype(bool)
    W = np.asarray(W, dtype=np.float32)
    bv = np.asarray(b, dtype=np.float32)

    pj = [np.nonzero(mask[s])[0] for s in range(BS)]
    aj = [np.nonzero(~mask[s])[0] for s in range(BS)]
    cPs = [len(x) for x in pj]
    cAs = [len(x) for x in aj]
    assert all(c > 0 for c in cPs), "a sample with zero present edges divides by zero"
    cAmax = max(1, max(cAs))
    CA = 2
    while CA < cAmax:
        CA *= 2

    perms = []
    for s in range(BS):
        pad_p = np.full(N - cPs[s], pj[s][0], dtype=np.int64)
        if cAs[s] > 0:
            tail = np.concatenate(
                [aj[s], np.full(CA - cAs[s], aj[s][0], dtype=np.int64)]
            )
        else:
            tail = np.full(CA, pj[s][0], dtype=np.int64)
        perms.append(np.concatenate([pj[s], pad_p, tail]))

    E16 = E.astype(np.float16)
    WT = np.ascontiguousarray(W.T).astype(np.float16)  # [FI, DX]
    ident = np.eye(P, dtype=np.float16)
    brow = np.ascontiguousarray(bv.reshape(1, DX)).astype(np.float16)

    if CA not in _NC_CACHE:
        _NC_CACHE[CA] = build_program(CA)
    nc = _NC_CACHE[CA]

    in_maps = []
    for c in range(NCORES):
        egs = np.empty((BPC * N, N + CA, DE), np.float16)
        scal = np.empty((NBLK, P, 3), np.float32)
        for bl in range(BPC):
            s = c * BPC + bl
            egs[bl * N : (bl + 1) * N] = E16[s][:, perms[s], :]
            npadA = (CA - cAs[s]) if cAs[s] > 0 else CA
            for ih in range(2):
                scal[bl * 2 + ih, :] = (
                    float(N - cPs[s]),
                    float(npadA),
                    -1.0 / cPs[s],
                )
        in_maps.append(
            {"eg": egs, "wt": WT, "brow": brow, "ident": ident, "scal": scal}
        )

    trace = os.environ.get("NN_KERNEL_TRACE", "0") == "1"
    if trace:
        _enable_tracing()
    res = run_bass_kernel_spmd(
        nc, in_maps, list(range(NCORES)), trace=trace, tmpdir="/tmp/nn_kernel_trace"
    )
    LAST_RESULT["exec_time_ns"] = res.exec_time_ns
    LAST_RESULT["mean_exec_time_ns"] = res.mean_exec_time_ns
    LAST_RESULT["profile_json"] = res.profile_json

    out = np.concatenate(
        [res.results[c]["out"].reshape(BPC, N, DX) for c in range(NCORES)], axis=0
    )
    return out.astype(np.float32)


# revision 15
# speedup vs baseline: 1.3100x; 1.3100x over previous
"""Trainium2 Bass kernel for masked edge pooling + linear (nn_EtoX).

Reference computation (per sample b, node i, over neighbors j with mask[b, j]):
  m   = sum_j E[b,i,j,:] / count_b          (unmasked sum / masked count)
  mi  = min over present j of E[b,i,j,:]
  ma  = max over present j of E[b,i,j,:]
  std = sum_{present j} (E - m)^2 / count_b
  out = concat(m, mi, ma, std) @ W.T + bias

Strategy (v2): data-parallel over batch (2 samples per core, 8 cores). The
host permutes each sample's j axis so present rows come first (padded to 256
with duplicates of the first present row), then absent rows (padded to CA
with duplicates of the first absent row), and uploads the result in FP16 —
halving HBM traffic and removing all on-device casts.

On-device per (sample, 128-row i-block), everything is pairwise fp16
tensor_tensor trees on the Vector engine (2x perf mode):
  - min/max trees over the 256 present rows
  - sum tree over present rows (pad contribution subtracted exactly)
  - ScalarE squares the present tile; sum tree over squares gives sumsq
  - sum tree over the CA absent rows (for the unmasked-mean correction)
  - small fused epilogue algebra forms m and std; TensorE transposes the
    packed z halves and applies the 256x256 linear (+bias) in fp16
"""

import os

# Whole-tile dependency granularity: lets a 1-element ACT "fence" write
# supersede a DMA-landed tile's reader/writer dep set, keeping every DMA
# instruction within the hardware's 2-sync-wait budget.
os.environ.setdefault("BY_DEFAULT_DISABLE_SUBTILE_DEPS", "1")

import numpy as np

try:
    from concourse import bass, mybir, tile
    from concourse.bass_utils import run_bass_kernel_spmd
except ImportError:  # fall back to the container's repo checkout
    import sys

    sys.path.insert(0, "/opt/trn_rl_repo")
    from concourse import bass, mybir, tile
    from concourse.bass_utils import run_bass_kernel_spmd

BS, N, DE, DX = 16, 256, 64, 256
FI = 4 * DE
NCORES = 8
BPC = BS // NCORES  # samples per core
P = 128
NBLK = BPC * 2  # 128-row i-blocks per core

F32 = mybir.dt.float32
F16 = mybir.dt.float16

LAST_RESULT = {}

_NC_CACHE = {}


def _enable_tracing():
    """Install the NTFF profile hook that the image's ``antenv`` lacks."""
    import contextlib
    import ctypes
    import sys
    import types

    try:
        import antenv.axon_hooks  # noqa: F401

        pass
    except ImportError:
        so_path = "/opt/axon/libaxon_pjrt.so"
        lib = ctypes.CDLL(so_path)
        if hasattr(lib, "axon_start_nrt_profile"):
            lib.axon_start_nrt_profile.argtypes = [
                ctypes.POINTER(ctypes.c_int64),
                ctypes.c_size_t,
            ]
            lib.axon_start_nrt_profile.restype = ctypes.c_int64
            lib.axon_stop_nrt_profile.argtypes = [ctypes.c_char_p]
            lib.axon_stop_nrt_profile.restype = ctypes.c_int64

            @contextlib.contextmanager
            def _hook(output_dir, device_ids):
                import jax

                jax.devices()
                if device_ids:
                    ids = (ctypes.c_int64 * len(device_ids))(*device_ids)
                    rc = lib.axon_start_nrt_profile(ids, len(device_ids))
                else:
                    rc = lib.axon_start_nrt_profile(None, 0)
                if rc != 0:
                    raise RuntimeError(f"axon_start_nrt_profile rc={rc}")
                try:
                    yield
                finally:
                    n = lib.axon_stop_nrt_profile(str(output_dir).encode())
                    print(f"profile: {n} file(s) written to {output_dir}")

            mod = types.ModuleType("antenv.axon_hooks")
            mod.get_axon_ntff_profile_hook = lambda: _hook
            mod.set_axon_ntff_profile_hook = lambda h: None
            import antenv

            sys.modules["antenv.axon_hooks"] = mod
            antenv.axon_hooks = mod

    from concourse import bass_utils as _bu

    _bu.upload_artifacts = lambda tmpdir: f"file://{tmpdir}"


def _hoist_excess_waits(bir: dict) -> dict:
    """Walrus (this build) rejects instructions whose embedded sync-wait list
    exceeds the ISA struct's slots. Hoist all but one wait into standalone
    single-wait EventSemaphore instructions placed immediately before the
    instruction on the same engine stream - semantically identical (sequencer
    executes waits in stream order before the op)."""
    ctr = 0
    for fn in bir["functions"]:
        for blk in fn["blocks"]:
            new = []
            for ins in blk["instructions"]:
                si = ins.get("sync_info")
                if si:
                    waits = si.get("on_wait") or []
                    if len(waits) > 1:
                        for w in waits[:-1]:
                            ctr += 1
                            new.append(
                                {
                                    "debug": ins.get("debug", 0),
                                    "engine": ins["engine"],
                                    "ins": [],
                                    "outs": [],
                                    "name": f"hoistw-{ctr}",
                                    "opcode": "EventSemaphore",
                                    "sync_info": {"on_update": [], "on_wait": [w]},
                                }
                            )
                        si["on_wait"] = [waits[-1]]
                new.append(ins)
            blk["instructions"] = new
    return bir


def build_program(CA: int) -> "bass.Bass":
    nc = bass.Bass()
    NI = BPC * N  # flattened (sample, i) rows
    eg = nc.declare_dram_parameter("eg", [NI, N + CA, DE], F16, isOutput=False)
    wt = nc.declare_dram_parameter("wt", [FI, DX], F16, isOutput=False)
    brow = nc.declare_dram_parameter("brow", [1, DX], F16, isOutput=False)
    ident = nc.declare_dram_parameter("ident", [P, P], F16, isOutput=False)
    scal = nc.declare_dram_parameter("scal", [NBLK, P, 3], F32, isOutput=False)
    out = nc.declare_dram_parameter("out", [NI, DX], F32, isOutput=True)

    MIN = mybir.AluOpType.min
    MAX = mybir.AluOpType.max
    ADD = mybir.AluOpType.add
    SUB = mybir.AluOpType.subtract
    MUL = mybir.AluOpType.mult
    SQUARE = mybir.ActivationFunctionType.Square

    with tile.TileContext(nc) as tc:
        with (
            tc.tile_pool(name="singles", bufs=1) as singles,
            tc.tile_pool(name="main", bufs=2) as mainp,
            tc.tile_pool(name="absp", bufs=2) as absp,
            tc.tile_pool(name="sqp", bufs=2) as sqp,
            tc.tile_pool(name="scr", bufs=1) as scr,
            tc.tile_pool(name="stats", bufs=2) as stats,
            tc.tile_pool(name="ep", bufs=2) as ep,
            tc.tile_pool(name="outp", bufs=2) as outp,
            tc.tile_pool(name="psum", bufs=2, space="PSUM") as psum,
        ):
            wt0 = singles.tile([P, DX], F16, tag="wt0")
            nc.sync.dma_start(out=wt0[:], in_=wt[0:P, :])
            wt1 = singles.tile([P, DX], F16, tag="wt1")
            nc.sync.dma_start(out=wt1[:], in_=wt[P:FI, :])
            id_t = singles.tile([P, P], F16, tag="id")
            nc.sync.dma_start(out=id_t[:], in_=ident[:, :])
            br_t = singles.tile([1, DX], F16, tag="br")
            nc.sync.dma_start(out=br_t[:], in_=brow[:, :])
            ones1 = singles.tile([1, P], F16, tag="ones")
            nc.vector.memset(ones1[:], 1.0)
            sc = {}
            for b in range(NBLK):
                for k, nm in enumerate(("npadP", "npadA", "negInvC")):
                    t = singles.tile([P, 1], F32, tag=f"sc{b}{k}")
                    nc.sync.dma_start(out=t[:], in_=scal[b, :, k : k + 1])
                    sc[(b, nm)] = t

            # shared tree scratch (bufs=1: chains serialize on DVE anyway)
            scratch = {}
            for w in (128, 64, 32, 16, 8, 4, 2):
                scratch[w] = scr.tile([P, w, DE], F16, tag=f"t{w}", name=f"t{w}")

            def chain(op, a_ap, b_ap, w1, final_dst, eng=None, sdict=None):
                """Pairwise reduce: L1 combines a_ap/b_ap ([P, w1, DE] each),
                then halve until one [P, DE] row written to final_dst."""
                eng = eng or nc.vector
                sdict = sdict or scratch
                cur = sdict[w1]
                eng.tensor_tensor(cur[:], a_ap, b_ap, op)
                w = w1 // 2
                while w >= 1:
                    if w == 1:
                        eng.tensor_tensor(
                            final_dst,
                            cur[:, 0:1, :].rearrange("p a d -> p (a d)"),
                            cur[:, 1:2, :].rearrange("p a d -> p (a d)"),
                            op,
                        )
                    else:
                        eng.tensor_tensor(
                            sdict[w][:], cur[:, 0:w, :], cur[:, w : 2 * w, :], op
                        )
                        cur = sdict[w]
                    w //= 2

            for b in range(NBLK):
                s, ih = b // 2, b % 2
                r0 = s * N + ih * P
                mainA = mainp.tile([P, P, DE], F16, tag="mainA")
                nc.sync.dma_start(out=mainA[:], in_=eg[r0 : r0 + P, 0:P, :])
                mainB = mainp.tile([P, P, DE], F16, tag="mainB")
                nc.sync.dma_start(out=mainB[:], in_=eg[r0 : r0 + P, P:N, :])
                abst = absp.tile([P, CA, DE], F16, tag="abs")
                nc.sync.dma_start(out=abst[:], in_=eg[r0 : r0 + P, N : N + CA, :])
                sqA = sqp.tile([P, P, DE], F16, tag="sqA")
                nc.scalar.activation(sqA[:], mainA[:], SQUARE)
                sqB = sqp.tile([P, P, DE], F16, tag="sqB")
                nc.scalar.activation(sqB[:], mainB[:], SQUARE)

                z01 = stats.tile([P, P], F16, tag="z01")  # [m | mi]
                z23 = stats.tile([P, P], F16, tag="z23")  # [ma | std]
                SPt = stats.tile([P, DE], F32, tag="SP")
                QPt = stats.tile([P, DE], F32, tag="QP")
                SAt = stats.tile([P, DE], F32, tag="SA")

                a_h, b_h = mainA[:, :, :], mainB[:, :, :]
                chain(MIN, a_h, b_h, P, z01[:, DE : 2 * DE])
                chain(MAX, a_h, b_h, P, z23[:, 0:DE])
                chain(ADD, a_h, b_h, P, SPt[:])
                chain(ADD, sqA[:, :, :], sqB[:, :, :], P, QPt[:])
                ch = CA // 2
                chain(ADD, abst[:, 0:ch, :], abst[:, ch:CA, :], ch, SAt[:])

                # epilogue: m = (S_p + S_a)/c ; std = (Q_p - m (S_p - S_a))/c
                # computed via negated partials so scalar_tensor_tensor fuses
                def et(tag):
                    return ep.tile([P, DE], F32, tag=tag, name=tag)

                x0 = mainA[:, 0, :]
                x0sq = sqA[:, 0, :]
                a0 = abst[:, 0, :]
                nSPc = et("nSPc")  # npadP*x0 - SP_tree = -S_p
                nc.vector.scalar_tensor_tensor(
                    nSPc[:], x0, sc[(b, "npadP")][:], SPt[:], MUL, SUB
                )
                nQPc = et("nQPc")  # npadP*x0^2 - QP_tree = -Q_p
                nc.vector.scalar_tensor_tensor(
                    nQPc[:], x0sq, sc[(b, "npadP")][:], QPt[:], MUL, SUB
                )
                nSAc = et("nSAc")  # npadA*a0 - SA_tree = -S_a
                nc.vector.scalar_tensor_tensor(
                    nSAc[:], a0, sc[(b, "npadA")][:], SAt[:], MUL, SUB
                )
                nS = et("nS")
                nc.vector.tensor_tensor(nS[:], nSPc[:], nSAc[:], ADD)
                m_dst = z01[:, 0:DE]
                nc.vector.tensor_scalar(m_dst, nS[:], sc[(b, "negInvC")][:], None, MUL)
                d_t = et("d")  # S_p - S_a
                nc.vector.tensor_tensor(d_t[:], nSAc[:], nSPc[:], SUB)
                e_t = et("e")
                nc.vector.tensor_tensor(e_t[:], m_dst, d_t[:], MUL)
                f_t = et("f")  # -(Q_p - m(S_p - S_a))
                nc.vector.tensor_tensor(f_t[:], nQPc[:], e_t[:], ADD)
                nc.vector.tensor_scalar(
                    z23[:, DE : 2 * DE], f_t[:], sc[(b, "negInvC")][:], None, MUL
                )

                # fences: collapse each DMA-landed tile's reader set
                nc.scalar.mul(mainA[0:1, 0:1, 0:1], mainA[0:1, 0:1, 0:1], 0.0)
                nc.scalar.mul(mainB[0:1, 0:1, 0:1], mainB[0:1, 0:1, 0:1], 0.0)
                nc.scalar.mul(sqA[0:1, 0:1, 0:1], sqA[0:1, 0:1, 0:1], 0.0)
                nc.scalar.mul(sqB[0:1, 0:1, 0:1], sqB[0:1, 0:1, 0:1], 0.0)
                nc.scalar.mul(abst[0:1, 0:1, 0:1], abst[0:1, 0:1, 0:1], 0.0)

                # transpose packed stats into z^T layout ([feature, i])
                psz0 = psum.tile([P, P], F16, tag="psz0")
                nc.tensor.transpose(out=psz0[:], in_=z01[:], identity=id_t[:])
                psz1 = psum.tile([P, P], F16, tag="psz1")
                nc.tensor.transpose(out=psz1[:], in_=z23[:], identity=id_t[:])
                zT0 = outp.tile([P, P], F16, tag="zT0")
                nc.scalar.copy(out=zT0[:], in_=psz0[:])
                zT1 = outp.tile([P, P], F16, tag="zT1")
                nc.scalar.copy(out=zT1[:], in_=psz1[:])

                pso = psum.tile([P, DX], F32, tag="pso")
                nc.tensor.matmul(pso[:], zT0[:], wt0[:], start=True, stop=False)
                nc.tensor.matmul(pso[:], zT1[:], wt1[:], start=False, stop=False)
                nc.tensor.matmul(pso[:], ones1[:], br_t[:], start=False, stop=True)
                o_t = outp.tile([P, DX], F32, tag="o_t")
                nc.scalar.copy(out=o_t[:], in_=pso[:])
                nc.sync.dma_start(out=out[r0 : r0 + P, :], in_=o_t[:])

    import json as _json

    _orig_to_json = nc.to_json_bytes

    def _patched_to_json():
        return _json.dumps(_hoist_excess_waits(_json.loads(_orig_to_json()))).encode()

    nc.to_json_bytes = _patched_to_json
    return nc


def kernel(E, e_mask2, W, b):
    E = np.asarray(E, dtype=np.float32)
    mask = np.asarray(e_mask2).reshape(BS, N).astype(bool)
    W = np.asarray(W, dtype=np.float32)
    bv = np.asarray(b, dtype=np.float32)

    pj = [np.nonzero(mask[s])[0] for s in range(BS)]
    aj = [np.nonzero(~mask[s])[0] for s in range(BS)]
    cPs = [len(x) for x in pj]
    cAs = [len(x) for x in aj]
    assert all(c > 0 for c in cPs), "a sample with zero present edges divides by zero"
    cAmax = max(1, max(cAs))
    CA = 2
    while CA < cAmax:
        CA *= 2

    perms = []
    for s in range(BS):
        pad_p = np.full(N - cPs[s], pj[s][0], dtype=np.int64)
        if cAs[s] > 0:
            tail = np.concatenate(
                [aj[s], np.full(CA - cAs[s], aj[s][0], dtype=np.int64)]
            )
        else:
            tail = np.full(CA, pj[s][0], dtype=np.int64)
        perms.append(np.concatenate([pj[s], pad_p, tail]))

    E16 = E.astype(np.float16)
    WT = np.ascontiguousarray(W.T).astype(np.float16)  # [FI, DX]
    ident = np.eye(P, dtype=np.float16)
    brow = np.ascontiguousarray(bv.reshape(1, DX)).astype(np.float16)

    if CA not in _NC_CACHE:
        _NC_CACHE[CA] = build_program(CA)
    nc = _NC_CACHE[CA]

    in_maps = []
    for c in range(NCORES):
        egs = np.empty((BPC * N, N + CA, DE), np.float16)
        scal = np.empty((NBLK, P, 3), np.float32)
        for bl in range(BPC):
            s = c * BPC + bl
            egs[bl * N : (bl + 1) * N] = E16[s][:, perms[s], :]
            npadA = (CA - cAs[s]) if cAs[s] > 0 else CA
            for ih in range(2):
                scal[bl * 2 + ih, :] = (
                    float(N - cPs[s]),
                    float(npadA),
                    -1.0 / cPs[s],
                )
        in_maps.append(
            {"eg": egs, "wt": WT, "brow": brow, "ident": ident, "scal": scal}
        )

    trace = os.environ.get("NN_KERNEL_TRACE", "0") == "1"
    if trace:
        _enable_tracing()
    res = run_bass_kernel_spmd(
        nc, in_maps, list(range(NCORES)), trace=trace, tmpdir="/tmp/nn_kernel_trace"
    )
    LAST_RESULT["exec_time_ns"] = res.exec_time_ns
    LAST_RESULT["mean_exec_time_ns"] = res.mean_exec_time_ns
    LAST_RESULT["profile_json"] = res.profile_json

    out = np.concatenate(
        [res.results[c]["out"].reshape(BPC, N, DX) for c in range(NCORES)], axis=0
    )
    return out.astype(np.float32)


# revision 23
# speedup vs baseline: 1.5936x; 1.2165x over previous
"""Trainium2 Bass kernel for masked edge pooling + linear (nn_EtoX).

Reference computation (per sample b, node i, over neighbors j with mask[b, j]):
  m   = sum_j E[b,i,j,:] / count_b          (unmasked sum / masked count)
  mi  = min over present j of E[b,i,j,:]
  ma  = max over present j of E[b,i,j,:]
  std = sum_{present j} (E - m)^2 / count_b
  out = concat(m, mi, ma, std) @ W.T + bias

Strategy (v2): data-parallel over batch (2 samples per core, 8 cores). The
host permutes each sample's j axis so present rows come first (padded to 256
with duplicates of the first present row), then absent rows (padded to CA
with duplicates of the first absent row), and uploads the result in FP16 —
halving HBM traffic and removing all on-device casts.

On-device per (sample, 128-row i-block), everything is pairwise fp16
tensor_tensor trees on the Vector engine (2x perf mode):
  - min/max trees over the 256 present rows
  - sum tree over present rows (pad contribution subtracted exactly)
  - ScalarE squares the present tile; sum tree over squares gives sumsq
  - sum tree over the CA absent rows (for the unmasked-mean correction)
  - small fused epilogue algebra forms m and std; TensorE transposes the
    packed z halves and applies the 256x256 linear (+bias) in fp16
"""

import os

# Whole-tile dependency granularity: lets a 1-element ACT "fence" write
# supersede a DMA-landed tile's reader/writer dep set, keeping every DMA
# instruction within the hardware's 2-sync-wait budget.
os.environ.setdefault("BY_DEFAULT_DISABLE_SUBTILE_DEPS", "1")

import numpy as np

try:
    from concourse import bass, mybir, tile
    from concourse.bass_utils import run_bass_kernel_spmd
except ImportError:  # fall back to the container's repo checkout
    import sys

    sys.path.insert(0, "/opt/trn_rl_repo")
    from concourse import bass, mybir, tile
    from concourse.bass_utils import run_bass_kernel_spmd

BS, N, DE, DX = 16, 256, 64, 256
FI = 4 * DE
NCORES = 8
BPC = BS // NCORES  # samples per core
P = 128
NBLK = BPC * 2  # 128-row i-blocks per core

F32 = mybir.dt.float32
F16 = mybir.dt.float16

LAST_RESULT = {}

_NC_CACHE = {}


def _enable_tracing():
    """Install the NTFF profile hook that the image's ``antenv`` lacks."""
    import contextlib
    import ctypes
    import sys
    import types

    try:
        import antenv.axon_hooks  # noqa: F401

        pass
    except ImportError:
        so_path = "/opt/axon/libaxon_pjrt.so"
        lib = ctypes.CDLL(so_path)
        if hasattr(lib, "axon_start_nrt_profile"):
            lib.axon_start_nrt_profile.argtypes = [
                ctypes.POINTER(ctypes.c_int64),
                ctypes.c_size_t,
            ]
            lib.axon_start_nrt_profile.restype = ctypes.c_int64
            lib.axon_stop_nrt_profile.argtypes = [ctypes.c_char_p]
            lib.axon_stop_nrt_profile.restype = ctypes.c_int64

            @contextlib.contextmanager
            def _hook(output_dir, device_ids):
                import jax

                jax.devices()
                if device_ids:
                    ids = (ctypes.c_int64 * len(device_ids))(*device_ids)
                    rc = lib.axon_start_nrt_profile(ids, len(device_ids))
                else:
                    rc = lib.axon_start_nrt_profile(None, 0)
                if rc != 0:
                    raise RuntimeError(f"axon_start_nrt_profile rc={rc}")
                try:
                    yield
                finally:
                    n = lib.axon_stop_nrt_profile(str(output_dir).encode())
                    print(f"profile: {n} file(s) written to {output_dir}")

            mod = types.ModuleType("antenv.axon_hooks")
            mod.get_axon_ntff_profile_hook = lambda: _hook
            mod.set_axon_ntff_profile_hook = lambda h: None
            import antenv

            sys.modules["antenv.axon_hooks"] = mod
            antenv.axon_hooks = mod

    from concourse import bass_utils as _bu

    _bu.upload_artifacts = lambda tmpdir: f"file://{tmpdir}"


def _hoist_excess_waits(bir: dict) -> dict:
    """Walrus (this build) rejects instructions whose embedded sync-wait list
    exceeds the ISA struct's slots. Hoist all but one wait into standalone
    single-wait EventSemaphore instructions placed immediately before the
    instruction on the same engine stream - semantically identical (sequencer
    executes waits in stream order before the op)."""
    ctr = 0
    for fn in bir["functions"]:
        for blk in fn["blocks"]:
            new = []
            for ins in blk["instructions"]:
                si = ins.get("sync_info")
                if si:
                    waits = si.get("on_wait") or []
                    if len(waits) > 1:
                        for w in waits[:-1]:
                            ctr += 1
                            new.append(
                                {
                                    "debug": ins.get("debug", 0),
                                    "engine": ins["engine"],
                                    "ins": [],
                                    "outs": [],
                                    "name": f"hoistw-{ctr}",
                                    "opcode": "EventSemaphore",
                                    "sync_info": {"on_update": [], "on_wait": [w]},
                                }
                            )
                        si["on_wait"] = [waits[-1]]
                new.append(ins)
            blk["instructions"] = new
    return bir


def build_program(CA: int) -> "bass.Bass":
    nc = bass.Bass()
    NI = BPC * N  # flattened (sample, i) rows
    eg = nc.declare_dram_parameter("eg", [NI, N + CA, DE], F16, isOutput=False)
    wt = nc.declare_dram_parameter("wt", [FI, DX], F16, isOutput=False)
    brow = nc.declare_dram_parameter("brow", [1, DX], F16, isOutput=False)
    ident = nc.declare_dram_parameter("ident", [P, P], F16, isOutput=False)
    scal = nc.declare_dram_parameter("scal", [NBLK, P, 3], F32, isOutput=False)
    out = nc.declare_dram_parameter("out", [NI, DX], F32, isOutput=True)

    MIN = mybir.AluOpType.min
    MAX = mybir.AluOpType.max
    ADD = mybir.AluOpType.add
    SUB = mybir.AluOpType.subtract
    MUL = mybir.AluOpType.mult
    SQUARE = mybir.ActivationFunctionType.Square

    with tile.TileContext(nc) as tc:
        with (
            tc.tile_pool(name="singles", bufs=1) as singles,
            tc.tile_pool(name="main", bufs=2) as mainp,
            tc.tile_pool(name="absp", bufs=2) as absp,
            tc.tile_pool(name="sqp", bufs=2) as sqp,
            tc.tile_pool(name="scr", bufs=1) as scr,
            tc.tile_pool(name="stats", bufs=2) as stats,
            tc.tile_pool(name="ep", bufs=2) as ep,
            tc.tile_pool(name="outp", bufs=2) as outp,
            tc.tile_pool(name="psum", bufs=2, space="PSUM") as psum,
        ):
            # block-0 data DMAs first so compute starts ASAP (the singles
            # below share the sync queue but aren't needed until later)
            main_b0 = mainp.tile([P, N, DE], F16, tag="main")
            nc.sync.dma_start(out=main_b0[:], in_=eg[0:P, 0:N, :])
            abst_b0 = absp.tile([P, CA, DE], F16, tag="abs")
            nc.scalar.dma_start(out=abst_b0[:], in_=eg[0:P, N : N + CA, :])

            wt0 = singles.tile([P, DX], F16, tag="wt0")
            nc.sync.dma_start(out=wt0[:], in_=wt[0:P, :])
            wt1 = singles.tile([P, DX], F16, tag="wt1")
            nc.sync.dma_start(out=wt1[:], in_=wt[P:FI, :])
            id_t = singles.tile([P, P], F16, tag="id")
            nc.sync.dma_start(out=id_t[:], in_=ident[:, :])
            br_t = singles.tile([1, DX], F16, tag="br")
            nc.sync.dma_start(out=br_t[:], in_=brow[:, :])
            ones1 = singles.tile([1, P], F16, tag="ones")
            nc.vector.memset(ones1[:], 1.0)
            sc = {}
            for b in range(NBLK):
                for k, nm in enumerate(("npadP", "npadA", "negInvC")):
                    t = singles.tile([P, 1], F32, tag=f"sc{b}{k}")
                    nc.sync.dma_start(out=t[:], in_=scal[b, :, k : k + 1])
                    sc[(b, nm)] = t

            # shared tree scratch (bufs=1: chains serialize on DVE anyway)
            scratch = {}
            for w in (128, 64, 32, 16, 8, 4, 2):
                scratch[w] = scr.tile([P, w, DE], F16, tag=f"t{w}", name=f"t{w}")

            def chain(op, a_ap, b_ap, w1, final_dst, eng=None, sdict=None):
                """Pairwise reduce: L1 combines a_ap/b_ap ([P, w1, DE] each),
                then halve until one [P, DE] row written to final_dst."""
                eng = eng or nc.vector
                sdict = sdict or scratch
                cur = sdict[w1]
                eng.tensor_tensor(cur[:], a_ap, b_ap, op)
                w = w1 // 2
                while w >= 1:
                    if w == 1:
                        eng.tensor_tensor(
                            final_dst,
                            cur[:, 0:1, :].rearrange("p a d -> p (a d)"),
                            cur[:, 1:2, :].rearrange("p a d -> p (a d)"),
                            op,
                        )
                    else:
                        eng.tensor_tensor(
                            sdict[w][:], cur[:, 0:w, :], cur[:, w : 2 * w, :], op
                        )
                        cur = sdict[w]
                    w //= 2

            for b in range(NBLK):
                s, ih = b // 2, b % 2
                r0 = s * N + ih * P
                if b == 0:
                    main, abst = main_b0, abst_b0
                else:
                    main = mainp.tile([P, N, DE], F16, tag="main")
                    nc.sync.dma_start(out=main[:], in_=eg[r0 : r0 + P, 0:N, :])
                    abst = absp.tile([P, CA, DE], F16, tag="abs")
                    nc.scalar.dma_start(out=abst[:], in_=eg[r0 : r0 + P, N : N + CA, :])
                sq = sqp.tile([P, N, DE], F16, tag="sq")
                nc.scalar.activation(sq[:], main[:], SQUARE)
                mainA = main[:, 0:P, :]
                mainB = main[:, P:N, :]
                sqA = sq[:, 0:P, :]
                sqB = sq[:, P:N, :]

                z01 = stats.tile([P, P], F16, tag="z01")  # [m | mi]
                z23 = stats.tile([P, P], F16, tag="z23")  # [ma | std]
                SPt = stats.tile([P, DE], F32, tag="SP")
                QPt = stats.tile([P, DE], F32, tag="QP")
                SAt = stats.tile([P, DE], F32, tag="SA")

                a_h, b_h = mainA, mainB
                chain(MIN, a_h, b_h, P, z01[:, DE : 2 * DE])
                chain(MAX, a_h, b_h, P, z23[:, 0:DE])
                chain(ADD, a_h, b_h, P, SPt[:])
                chain(ADD, sqA, sqB, P, QPt[:])
                ch = CA // 2
                chain(ADD, abst[:, 0:ch, :], abst[:, ch:CA, :], ch, SAt[:])

                # epilogue: m = (S_p + S_a)/c ; std = (Q_p - m (S_p - S_a))/c
                # computed via negated partials so scalar_tensor_tensor fuses
                def et(tag):
                    return ep.tile([P, DE], F32, tag=tag, name=tag)

                x0 = main[:, 0, :]
                x0sq = sq[:, 0, :]
                a0 = abst[:, 0, :]
                nSPc = et("nSPc")  # npadP*x0 - SP_tree = -S_p
                nc.vector.scalar_tensor_tensor(
                    nSPc[:], x0, sc[(b, "npadP")][:], SPt[:], MUL, SUB
                )
                nQPc = et("nQPc")  # npadP*x0^2 - QP_tree = -Q_p
                nc.vector.scalar_tensor_tensor(
                    nQPc[:], x0sq, sc[(b, "npadP")][:], QPt[:], MUL, SUB
                )
                nSAc = et("nSAc")  # npadA*a0 - SA_tree = -S_a
                nc.vector.scalar_tensor_tensor(
                    nSAc[:], a0, sc[(b, "npadA")][:], SAt[:], MUL, SUB
                )
                nS = et("nS")
                nc.vector.tensor_tensor(nS[:], nSPc[:], nSAc[:], ADD)
                m_dst = z01[:, 0:DE]
                nc.vector.tensor_scalar(m_dst, nS[:], sc[(b, "negInvC")][:], None, MUL)
                d_t = et("d")  # S_p - S_a
                nc.vector.tensor_tensor(d_t[:], nSAc[:], nSPc[:], SUB)
                e_t = et("e")
                nc.vector.tensor_tensor(e_t[:], m_dst, d_t[:], MUL)
                f_t = et("f")  # -(Q_p - m(S_p - S_a))
                nc.vector.tensor_tensor(f_t[:], nQPc[:], e_t[:], ADD)
                nc.vector.tensor_scalar(
                    z23[:, DE : 2 * DE], f_t[:], sc[(b, "negInvC")][:], None, MUL
                )

                # fences: collapse each DMA-landed tile's reader set
                nc.scalar.mul(main[0:1, 0:1, 0:1], main[0:1, 0:1, 0:1], 0.0)
                nc.scalar.mul(sq[0:1, 0:1, 0:1], sq[0:1, 0:1, 0:1], 0.0)
                nc.scalar.mul(abst[0:1, 0:1, 0:1], abst[0:1, 0:1, 0:1], 0.0)

                # transpose packed stats into z^T layout ([feature, i])
                psz0 = psum.tile([P, P], F16, tag="psz0")
                nc.tensor.transpose(out=psz0[:], in_=z01[:], identity=id_t[:])
                psz1 = psum.tile([P, P], F16, tag="psz1")
                nc.tensor.transpose(out=psz1[:], in_=z23[:], identity=id_t[:])
                zT0 = outp.tile([P, P], F16, tag="zT0")
                nc.scalar.copy(out=zT0[:], in_=psz0[:])
                zT1 = outp.tile([P, P], F16, tag="zT1")
                nc.scalar.copy(out=zT1[:], in_=psz1[:])

                pso = psum.tile([P, DX], F32, tag="pso")
                nc.tensor.matmul(pso[:], zT0[:], wt0[:], start=True, stop=False)
                nc.tensor.matmul(pso[:], zT1[:], wt1[:], start=False, stop=False)
                nc.tensor.matmul(pso[:], ones1[:], br_t[:], start=False, stop=True)
                o_t = outp.tile([P, DX], F32, tag="o_t")
                nc.scalar.copy(out=o_t[:], in_=pso[:])
                nc.sync.dma_start(out=out[r0 : r0 + P, :], in_=o_t[:])

    import json as _json

    _orig_to_json = nc.to_json_bytes

    def _patched_to_json():
        return _json.dumps(_hoist_excess_waits(_json.loads(_orig_to_json()))).encode()

    nc.to_json_bytes = _patched_to_json
    return nc


def kernel(E, e_mask2, W, b):
    E = np.asarray(E, dtype=np.float32)
    mask = np.asarray(e_mask2).reshape(BS, N).astype(bool)
    W = np.asarray(W, dtype=np.float32)
    bv = np.asarray(b, dtype=np.float32)

    pj = [np.nonzero(mask[s])[0] for s in range(BS)]
    aj = [np.nonzero(~mask[s])[0] for s in range(BS)]
    cPs = [len(x) for x in pj]
    cAs = [len(x) for x in aj]
    assert all(c > 0 for c in cPs), "a sample with zero present edges divides by zero"
    cAmax = max(1, max(cAs))
    CA = 2
    while CA < cAmax:
        CA *= 2

    perms = []
    for s in range(BS):
        pad_p = np.full(N - cPs[s], pj[s][0], dtype=np.int64)
        if cAs[s] > 0:
            tail = np.concatenate(
                [aj[s], np.full(CA - cAs[s], aj[s][0], dtype=np.int64)]
            )
        else:
            tail = np.full(CA, pj[s][0], dtype=np.int64)
        perms.append(np.concatenate([pj[s], pad_p, tail]))

    E16 = E.astype(np.float16)
    WT = np.ascontiguousarray(W.T).astype(np.float16)  # [FI, DX]
    ident = np.eye(P, dtype=np.float16)
    brow = np.ascontiguousarray(bv.reshape(1, DX)).astype(np.float16)

    if CA not in _NC_CACHE:
        _NC_CACHE[CA] = build_program(CA)
    nc = _NC_CACHE[CA]

    in_maps = []
    for c in range(NCORES):
        egs = np.empty((BPC * N, N + CA, DE), np.float16)
        scal = np.empty((NBLK, P, 3), np.float32)
        for bl in range(BPC):
            s = c * BPC + bl
            egs[bl * N : (bl + 1) * N] = E16[s][:, perms[s], :]
            npadA = (CA - cAs[s]) if cAs[s] > 0 else CA
            for ih in range(2):
                scal[bl * 2 + ih, :] = (
                    float(N - cPs[s]),
                    float(npadA),
                    -1.0 / cPs[s],
                )
        in_maps.append(
            {"eg": egs, "wt": WT, "brow": brow, "ident": ident, "scal": scal}
        )

    trace = os.environ.get("NN_KERNEL_TRACE", "0") == "1"
    if trace:
        _enable_tracing()
    res = run_bass_kernel_spmd(
        nc, in_maps, list(range(NCORES)), trace=trace, tmpdir="/tmp/nn_kernel_trace"
    )
    LAST_RESULT["exec_time_ns"] = res.exec_time_ns
    LAST_RESULT["mean_exec_time_ns"] = res.mean_exec_time_ns
    LAST_RESULT["profile_json"] = res.profile_json

    out = np.concatenate(
        [res.results[c]["out"].reshape(BPC, N, DX) for c in range(NCORES)], axis=0
    )
    return out.astype(np.float32)
